# revision 1
# baseline (speedup 1.0000x reference)
"""Trainium2 Bass kernel for nn_Colar_static (retrieval_knn).

Sharding: data-parallel over batch B=2048 across 8 NeuronCores (256 rows each).
Static exemplar banks and weights are precomputed/reshaped on host and
replicated to all cores.

Per-core pipeline (all layouts keep batch in the matmul FREE dim or on
partitions as needed; j = flattened (class, exemplar) = 21*32 = 672):
  1. kvT[o,b]   = WkvT-blocks^T @ xT          (PE, bf16, K=2048)
  2. kT,vT      = psum evict (+bias, relu for v)  (ACT)
  3. sumsq[b]   = ones-matmul over kT^2       (DVE square + PE)
     rinv[b]    = 1/sqrt(sumsq)               (ACT sqrt + DVE recip)
  4. dot[b,j]   = kT-blocks^T @ Ekn_mat       (PE, K=1024)
  5. e = exp(rinv*dot)  (ACT, per-partition scale; cos in [-1,1] so no
     max-subtraction is needed for softmax stability)
  6. S,num      = blockwise reduces over 32-exemplar groups (DVE)
     t = num/S; g = exp(t); fw = g/G; c = fw/S  (class softmax; the scalar
     bias bw cancels in softmax)
  7. u[b,j] = e * c (block-broadcast)         (DVE)
  8. uT = PE-transpose(u)                     (PE + copies)
  9. fE_T[o,b]  = A_mat-blocks^T @ uT         (PE, K=672), relu evict
 10. outT[cls,b]= WoutT-blocks^T @ [relu(vT); relu(fE_T)]  (PE, K=2048)
 11. out = psum + bout -> DMA                 (DVE)

Host gathers the 8 [21,256] results into [2048, 21, 1].
"""

import numpy as np
import ml_dtypes

import concourse.bass as bass
import concourse.bacc as bacc
import concourse.mybir as mybir
import concourse.tile as tile
from concourse.bass_utils import run_bass_kernel_spmd

AF = mybir.ActivationFunctionType
BF = mybir.dt.bfloat16
F32 = mybir.dt.float32
bf16 = ml_dtypes.bfloat16

# Problem constants (hardcoded; kernel.py must be self-contained)
B, T, CIN, CH, M, NCLS = 2048, 8, 2048, 1024, 32, 21
NCORES = 8
BL = B // NCORES          # 256 batch rows per core
J = NCLS * M              # 672
P = 128
KB = CIN // P             # 16 contraction blocks for kv
OB = 2 * CH // P          # 16 output-channel blocks for kv
KHB = CH // P             # 8 blocks of k/v half
JBS = [P] * 5 + [J - 5 * P]   # j blocks: 5x128 + 32
NB = BL // P              # 2 batch chunks of 128


def build_nc(debug=False, repeat=1):
    nc = bacc.Bacc("TRN2", target_bir_lowering=False, debug=debug,
                   num_devices=NCORES)

    # all inputs are shipped in the exact per-partition SBUF layout so every
    # DMA is a plain [128, N]-contiguous copy (max DMA efficiency)
    xt_e = nc.dram_tensor("xt", [P, KB * BL], BF, kind="ExternalInput")
    wkv_e = nc.dram_tensor("wkv", [OB, P, KB * P], BF, kind="ExternalInput")
    ekn_e = nc.dram_tensor("ekn", [P, KHB * J], BF, kind="ExternalInput")
    amat_e = nc.dram_tensor("amat", [P, 6 * CH], BF, kind="ExternalInput")
    evwb_e = nc.dram_tensor("evwb", [P, J], BF, kind="ExternalInput")
    wout_e = nc.dram_tensor("wout", [P, KB * NCLS], BF, kind="ExternalInput")
    bkv_e = nc.dram_tensor("bkv", [P, OB], F32, kind="ExternalInput")
    bout_e = nc.dram_tensor("bout", [NCLS, 1], F32, kind="ExternalInput")
    ident_e = nc.dram_tensor("ident", [P, P], BF, kind="ExternalInput")
    out_e = nc.dram_tensor("out", [NCLS, BL], F32, kind="ExternalOutput")

    with tile.TileContext(nc) as tc:
        from contextlib import ExitStack
        with ExitStack() as ctx:
            pers = ctx.enter_context(tc.tile_pool(name="pers", bufs=1))
            # ALL psum pools co-resident (1+2+2+1+2 = 8 banks) so no phase
            # ever waits on a pool-scope boundary; Tile interleaves freely.
            pmisc = ctx.enter_context(tc.tile_pool(name="pmisc", bufs=1, space="PSUM"))
            pkv = ctx.enter_context(tc.tile_pool(name="pkv", bufs=2, space="PSUM"))
            pdot = ctx.enter_context(tc.tile_pool(name="pdot", bufs=1, space="PSUM"))
            ptr = ctx.enter_context(tc.tile_pool(name="ptr", bufs=1, space="PSUM"))
            pfe = ctx.enter_context(tc.tile_pool(name="pfe", bufs=1, space="PSUM"))

            # body emitted `repeat` times for delta-timing benchmarks
            # (tags make repeats share SBUF slots; WAR deps serialize them)
            for _rep in range(repeat):
              # ---- SBUF tiles ----
              bkv_s = pers.tile([P, OB], F32, tag="bkv")
              bout_s = pers.tile([NCLS, 1], F32, tag="bout")
              ident_s = pers.tile([P, P], BF, tag="ident")
              evwb_s = pers.tile([P, J], BF, tag="evwb")
              ones_s = pers.tile([P, 1], BF, tag="ones")
              scratch_s = pers.tile([1, 1], F32, tag="scratch")
              xt_s = pers.tile([P, KB * BL], BF, tag="xt")
              wkv_s = pers.tile([P, OB * KB * P], BF, tag="wkv")
              ekn_s = pers.tile([P, KHB * J], BF, tag="ekn")
              a_s = pers.tile([P, 6 * CH], BF, tag="amat")
              wout_s = pers.tile([P, KB * NCLS], BF, tag="wout")
              kt_s = pers.tile([P, KHB * BL], BF, tag="kt")
              ksq_s = pers.tile([P, KHB * BL], BF, tag="ksq")
              hv_s = pers.tile([P, KHB * BL], BF, tag="hv")
              hfe_s = pers.tile([P, KHB * BL], BF, tag="hfe")
              e_s = pers.tile([P, NB * J], BF, tag="e")
              tmp_s = pers.tile([P, J], BF, tag="tmp")
              u_s = pers.tile([P, NB * J], BF, tag="u")
              ut_s = pers.tile([P, 6 * BL], BF, tag="ut")
              rinv_s = pers.tile([P, NB], F32, tag="rinv")
              rs1_s = pers.tile([P, NB], F32, tag="rs1")
              rs2_s = pers.tile([P, NB], F32, tag="rs2")
              magic_s = pers.tile([P, 1], mybir.dt.int32, tag="magic")
              s_s = pers.tile([P, NB * NCLS], F32, tag="s")
              num_s = pers.tile([P, NB * NCLS], F32, tag="num")
              sinv_s = pers.tile([P, NB * NCLS], F32, tag="sinv")
              t_s = pers.tile([P, NB * NCLS], F32, tag="t")
              g_s = pers.tile([P, NB * NCLS], F32, tag="g")
              gg_s = pers.tile([P, NB], F32, tag="gg")
              ginv_s = pers.tile([P, NB], F32, tag="ginv")
              c1_s = pers.tile([P, NB * NCLS], F32, tag="c1")
              c_s = pers.tile([P, NB * NCLS], F32, tag="c")
              out_sb = pers.tile([NCLS, BL], F32, tag="outsb")

              # ---- DMA schedule ----
              # critical path first on the sync (HWDGE) queue: xt quarters,
              # then k-half weight chunks, ekn (dot), v-half chunks with
              # amat/wout slotted before the last two.
              XQ = 4
              qs = KB * BL // XQ
              nc.sync.dma_start(xt_s[:, 0:qs], xt_e.ap()[:, 0:qs])
              # first weight block right after the first xt quarter so PE can
              # start; remaining xt quarters arrive before k-step 4
              nc.sync.dma_start(wkv_s[:, 0:KB * P], wkv_e.ap()[0])
              for q in range(1, XQ):
                  nc.sync.dma_start(xt_s[:, q * qs:(q + 1) * qs],
                                    xt_e.ap()[:, q * qs:(q + 1) * qs])
              nc.gpsimd.dma_start(bkv_s[:], bkv_e.ap())
              nc.gpsimd.dma_start(bout_s[:], bout_e.ap())
              nc.gpsimd.dma_start(ident_s[:], ident_e.ap())
              nc.gpsimd.dma_start(evwb_s[:], evwb_e.ap())
              nc.vector.memset(ones_s[:], 1.0)
              nc.vector.memset(magic_s[:], 0x5f3759df)

              # dummy Exp as the FIRST ACT op pins the exp table set, which
              # also contains Identity/Relu (all ACT fns used here) -> exactly
              # one table load, executed while PE waits on the first weight DMA
              nc.vector.memset(scratch_s[:], 1.0)
              nc.scalar.activation(scratch_s[:], scratch_s[:], AF.Exp)

              # DMA engines are a shared resource: one consumption-ordered
              # stream beats split queues. ekn/amat/wout go last (consumed at
              # ~30/42/55us, all delivered in time).
              for oj in range(1, OB - 4):
                  nc.sync.dma_start(
                      wkv_s[:, oj * KB * P:(oj + 1) * KB * P], wkv_e.ap()[oj])
              nc.sync.dma_start(ekn_s[:], ekn_e.ap())
              for oj in range(OB - 4, OB - 2):
                  nc.sync.dma_start(
                      wkv_s[:, oj * KB * P:(oj + 1) * KB * P], wkv_e.ap()[oj])
              nc.sync.dma_start(a_s[:], amat_e.ap())
              for oj in range(OB - 2, OB):
                  nc.sync.dma_start(
                      wkv_s[:, oj * KB * P:(oj + 1) * KB * P], wkv_e.ap()[oj])
              nc.sync.dma_start(wout_s[:], wout_e.ap())

              # ---- phase 1: kvT = WkvT^T-blocks @ xT; evict k (+bias) / relu(v+bias) ----
              def kv_block(oj):
                  ps = pkv.tile([P, BL], F32, tag="pkv")
                  base = oj * KB * P
                  for i in range(KB):
                      nc.tensor.matmul(ps[:],
                                       wkv_s[:, base + i * P: base + (i + 1) * P],
                                       xt_s[:, i * BL:(i + 1) * BL],
                                       start=(i == 0), stop=(i == KB - 1))
                  if oj < KHB:
                      sl = slice(oj * BL, (oj + 1) * BL)
                      nc.scalar.activation(kt_s[:, sl], ps[:], AF.Identity,
                                           bias=bkv_s[:, oj:oj + 1])
                      nc.vector.tensor_mul(ksq_s[:, sl], kt_s[:, sl], kt_s[:, sl])
                  else:
                      o2 = oj - KHB
                      nc.scalar.activation(hv_s[:, o2 * BL:(o2 + 1) * BL], ps[:],
                                           AF.Relu, bias=bkv_s[:, oj:oj + 1])

              for oj in range(OB - 4):
                  kv_block(oj)

              # ---- phase 2: sumsq via ones-matmul; rinv = rsqrt on DVE ----
              ps2 = pmisc.tile([P, NB], F32, tag="misc")
              for bc in range(NB):
                  for i in range(KHB):
                      nc.tensor.matmul(ps2[:, bc:bc + 1],
                                       ksq_s[:, i * BL + bc * P: i * BL + bc * P + P],
                                       ones_s[:],
                                       start=(i == 0), stop=(i == KHB - 1))
                  # rinv = rsqrt(sumsq) fully on DVE (magic constant + 2
                  # Newton steps, rel err ~4e-6): no ACT table switches
                  sq = rs1_s[:, bc:bc + 1]
                  nc.vector.tensor_copy(sq, ps2[:, bc:bc + 1])
                  y = rinv_s[:, bc:bc + 1]
                  nc.vector.tensor_scalar(
                      y.bitcast(mybir.dt.int32), sq.bitcast(mybir.dt.int32),
                      1, None, op0=mybir.AluOpType.logical_shift_right)
                  nc.vector.tensor_tensor(
                      out=y.bitcast(mybir.dt.int32), in0=magic_s[:],
                      in1=y.bitcast(mybir.dt.int32),
                      op=mybir.AluOpType.subtract)
                  for _ in range(2):
                      t1 = rs2_s[:, bc:bc + 1]
                      nc.vector.tensor_mul(t1, y, y)
                      nc.vector.tensor_mul(t1, t1, sq)
                      nc.vector.tensor_scalar(t1, t1, -0.5, 1.5,
                                              op0=mybir.AluOpType.mult,
                                              op1=mybir.AluOpType.add)
                      nc.vector.tensor_mul(y, y, t1)

              # ---- phase 3 pieces ----
              def dots(bc):
                  psd = pdot.tile([P, J], F32, tag="pdot")
                  for i in range(KHB):
                      lhs = kt_s[:, i * BL + bc * P: i * BL + bc * P + P]
                      nc.tensor.matmul(psd[:, 0:512], lhs,
                                       ekn_s[:, i * J: i * J + 512],
                                       start=(i == 0), stop=(i == KHB - 1))
                      nc.tensor.matmul(psd[:, 512:J], lhs,
                                       ekn_s[:, i * J + 512:(i + 1) * J],
                                       start=(i == 0), stop=(i == KHB - 1))
                  return psd

              def softmax_chain(bc, psd):
                  e_sl = e_s[:, bc * J:(bc + 1) * J]
                  # exp evict in two halves so the next dots() WAR-waits only
                  # half as long on the psd read
                  nc.scalar.activation(e_sl[:, 0:512], psd[:, 0:512], AF.Exp,
                                       scale=rinv_s[:, bc:bc + 1])
                  nc.scalar.activation(e_sl[:, 512:J], psd[:, 512:J], AF.Exp,
                                       scale=rinv_s[:, bc:bc + 1])
                  e3 = e_sl.rearrange("p (n m) -> p n m", m=M)
                  ncls_sl = slice(bc * NCLS, (bc + 1) * NCLS)
                  s2 = s_s[:, ncls_sl]
                  nc.vector.reduce_sum(s2, e3, axis=mybir.AxisListType.X)
                  nc.vector.tensor_mul(tmp_s[:], e_sl, evwb_s[:])
                  nc.vector.reduce_sum(num_s[:, ncls_sl],
                                       tmp_s[:].rearrange("p (n m) -> p n m", m=M),
                                       axis=mybir.AxisListType.X)
                  nc.vector.reciprocal(sinv_s[:, ncls_sl], s2)
                  nc.vector.tensor_mul(t_s[:, ncls_sl], num_s[:, ncls_sl],
                                       sinv_s[:, ncls_sl])
                  nc.scalar.activation(g_s[:, ncls_sl], t_s[:, ncls_sl], AF.Exp)
                  nc.vector.reduce_sum(gg_s[:, bc:bc + 1], g_s[:, ncls_sl],
                                       axis=mybir.AxisListType.X)
                  nc.vector.reciprocal(ginv_s[:, bc:bc + 1], gg_s[:, bc:bc + 1])
                  nc.vector.tensor_mul(c1_s[:, ncls_sl], g_s[:, ncls_sl],
                                       sinv_s[:, ncls_sl])
                  nc.vector.tensor_scalar_mul(c_s[:, ncls_sl], c1_s[:, ncls_sl],
                                              ginv_s[:, bc:bc + 1])
                  c_b = bass.AP(c_s.tensor, c_s[:, ncls_sl].offset,
                                c_s[:, ncls_sl].ap + [[0, M]])
                  u3 = u_s[:, bc * J:(bc + 1) * J].rearrange("p (n m) -> p n m", m=M)
                  nc.vector.tensor_mul(u3, e3, c_b)

              # ---- phase 4+5 per batch chunk: transpose u, then fE matmuls
              # with 8 accumulators packed into two psum banks; the jb0-2
              # matmuls overlap the group-1 eviction copy on DVE ----
              def transpose_fe(bc):
                  def tgroup(g, grp):
                      pst = ptr.tile([P, 3 * P], BF, tag="ptr")
                      for t, jb in enumerate(grp):
                          w = JBS[jb]
                          nc.tensor.transpose(
                              pst[:w, t * P:(t + 1) * P],
                              u_s[:, bc * J + jb * P: bc * J + jb * P + w],
                              ident_s[:])
                      n = sum(1 for jb in grp if JBS[jb] == P)
                      base = ut_s[:, grp[0] * BL + bc * P: grp[0] * BL + bc * P + P]
                      dst = bass.AP(ut_s.tensor, base.offset,
                                    [base.ap[0], [BL, n], base.ap[1]])
                      nc.vector.tensor_copy(
                          dst, pst[:, 0:n * P].rearrange("p (n q) -> p n q", q=P))
                      if n < len(grp):
                          jb = grp[n]
                          w = JBS[jb]
                          nc.vector.tensor_copy(
                              ut_s[:w, jb * BL + bc * P: jb * BL + bc * P + P],
                              pst[:w, n * P:(n + 1) * P])
                  tgroup(0, (0, 1, 2))
                  tgroup(1, (3, 4, 5))
                  if bc < NB - 1:
                      return
                  # ---- phase 5: fE = A^T-blocks @ uT, full batch width;
                  # relu evicts alternate ACT/DVE so neither queue's
                  # per-instruction overhead rate-limits PE ----
                  for oj in range(KHB):
                      acc = pfe.tile([P, BL], F32, tag=f"pfe{oj % 2}")
                      for jb in range(6):
                          w = JBS[jb]
                          nc.tensor.matmul(
                              acc[:],
                              a_s[:w, jb * CH + oj * P: jb * CH + (oj + 1) * P],
                              ut_s[:w, jb * BL:(jb + 1) * BL],
                              start=(jb == 0), stop=(jb == 5))
                      dst = hfe_s[:, oj * BL:(oj + 1) * BL]
                      if oj % 2 == 0:
                          nc.scalar.activation(dst, acc[:], AF.Relu)
                      else:
                          nc.vector.tensor_scalar_max(dst, acc[:], 0.0)

              # kv blocks 12-15 are PE filler under the two softmax chains
              # (dots/exp/DVE chain latency would otherwise idle PE ~7us)
              psd0 = dots(0)
              softmax_chain(0, psd0)
              psd1 = dots(1)
              softmax_chain(1, psd1)
              kv_block(OB - 4)
              transpose_fe(0)
              kv_block(OB - 3)
              transpose_fe(1)
              kv_block(OB - 2)
              kv_block(OB - 1)

              # ---- phase 6: outT = WoutT^T-blocks @ [hv; hfe]; +bout; DMA out ----
              pso = pmisc.tile([NCLS, BL], F32, tag="misc")
              for i in range(KB):
                  h_s = hv_s if i < KHB else hfe_s
                  ii = i % KHB
                  nc.tensor.matmul(pso[:], wout_s[:, i * NCLS:(i + 1) * NCLS],
                                   h_s[:, ii * BL:(ii + 1) * BL],
                                   start=(i == 0), stop=(i == KB - 1))
              nc.vector.tensor_scalar_add(out_sb[:], pso[:], bout_s[:, 0:1])
              nc.sync.dma_start(out_e.ap(), out_sb[:])

    nc.compile()
    return nc


def host_prep(x, static_feat, Wk, bk, Wv, bv, WEk, bEk, WEv, bEv, Ww, bw,
              Wout, bout):
    """Host-side fp32 precompute + per-core input maps."""
    EPS = 1e-8
    f32 = np.float32
    x = np.asarray(x, f32)
    static_feat = np.asarray(static_feat, f32)

    Ek = np.einsum('oc,ncm->nom', np.asarray(WEk, f32), static_feat,
                   optimize=True) + np.asarray(bEk, f32)[None, :, None]
    Ev = np.einsum('oc,ncm->nom', np.asarray(WEv, f32), static_feat,
                   optimize=True) + np.asarray(bEv, f32)[None, :, None]
    Ekn = Ek / np.maximum(np.linalg.norm(Ek, axis=1, keepdims=True), EPS)
    Ekn_mat = Ekn.transpose(1, 0, 2).reshape(CH, J)          # [CH, 672]
    A_mat = Ev.transpose(0, 2, 1).reshape(J, CH)             # [672, CH]
    evwb = np.einsum('nom,o->nm', Ev, np.asarray(Ww, f32)[0]).reshape(J)

    WkvT = np.concatenate([np.asarray(Wk, f32), np.asarray(Wv, f32)], axis=0).T
    bkv = np.concatenate([np.asarray(bk, f32), np.asarray(bv, f32)])
    xT = np.ascontiguousarray(x[:, -1, :].T)                 # [CIN, B]

    # [OB, P, KB*P]: per-o-chunk, per-partition-linear
    wkv_h = np.ascontiguousarray(
        WkvT.reshape(KB, P, OB, P).transpose(2, 1, 0, 3).reshape(
            OB, P, KB * P)).astype(bf16)
    ekn_h = np.ascontiguousarray(
        Ekn_mat.reshape(KHB, P, J).transpose(1, 0, 2).reshape(
            P, KHB * J)).astype(bf16)
    a_pad = np.zeros((6 * P, CH), np.float32)
    a_pad[:J] = A_mat
    amat_h = np.ascontiguousarray(
        a_pad.reshape(6, P, CH).transpose(1, 0, 2).reshape(P, 6 * CH)).astype(bf16)
    evwb_h = np.ascontiguousarray(
        np.broadcast_to(evwb.astype(bf16)[None, :], (P, J)))
    wout_h = np.ascontiguousarray(
        np.asarray(Wout, f32).T.reshape(KB, P, NCLS).transpose(1, 0, 2).reshape(
            P, KB * NCLS)).astype(bf16)
    bkv_h = np.ascontiguousarray(bkv.reshape(OB, P).T)
    bout_h = np.asarray(bout, f32).reshape(NCLS, 1)
    ident_h = np.eye(P, dtype=bf16)

    shared = dict(wkv=wkv_h, ekn=ekn_h, amat=amat_h, evwb=evwb_h,
                  wout=wout_h, bkv=bkv_h, bout=bout_h, ident=ident_h)
    in_maps = []
    for c in range(NCORES):
        xt_h = np.ascontiguousarray(
            xT[:, c * BL:(c + 1) * BL].reshape(KB, P, BL).transpose(1, 0, 2)
            .reshape(P, KB * BL)).astype(bf16)
        in_maps.append(dict(xt=xt_h, **shared))
    return in_maps


_NC_CACHE = {}


def get_nc(debug=False, repeat=1):
    key = (debug, repeat)
    if key not in _NC_CACHE:
        _NC_CACHE[key] = build_nc(debug=debug, repeat=repeat)
    return _NC_CACHE[key]


def kernel(**inputs) -> np.ndarray:
    nc = get_nc()
    in_maps = host_prep(**inputs)
    res = run_bass_kernel_spmd(nc, in_maps, list(range(NCORES)))
    out = np.empty((B, NCLS, 1), dtype=np.float32)
    for c in range(NCORES):
        out[c * BL:(c + 1) * BL, :, 0] = res.results[c]["out"].T
    return out



# revision 22
# speedup vs baseline: 1.4523x; 1.4523x over previous
"""Trainium2 Bass kernel for nn_Colar_static (retrieval_knn).

Sharding: data-parallel over batch B=2048 across 8 NeuronCores (256 rows each).
Weights/exemplars replicated, precomputed + quantized on host.

Design (vs the bf16 baseline at 53.3us):
  * Every large matmul is fp8e4m3 with the DoubleRow perf mode (K=256 per
    instruction, 0.5 cycles/row) -> 4x bf16 MAC rate and 1-byte weights
    (the kernel is DMA-bound: all DMA serializes at ~332 GB/s).
  * dots = x @ (Wk^T Ekn) directly: Wk is folded into the exemplars on the
    host, so the 2MB Wk and 0.7MB Ekn never ship; only D8 [CIN,672] (1.4MB).
  * ||k|| (softmax temperature only) via a random sketch: ||S k|| with
    S [128,1024] Gaussian, W_sk = S Wk [128, CIN] fp8 (0.25MB). The 5% norm
    error is invisible downstream (validated: rel err 3.21e-3, same as the
    exact-norm pipeline, because cos logits are tiny and softmax-smoothed).
  * v  = x8@Wv8hi + x8lo@Wv8hi + x8@Wv8lo   3-pass residual-compensated fp8
    (v dominates the output; plain fp8 fails at 3.3e-2).
  * fE = A8^T @ ut8 (fp8 DR);  out = Wout^T @ [hv;hfe] in bf16 (tiny).

Scales (all folded, no extra device work): D,W_sk x64; Wv x32; A x16; u x256.
The sketch scale cancels: rinv = rsqrt(sum((64 S k)^2)) = 1/(64||Sk||) and
dots are x64, so exp(dots*rinv) = exp(cos).

Rel err vs fp32 reference ~3.2e-3 (numpy-sim validated; gate is 2e-2).
"""

import numpy as np
import ml_dtypes

import concourse.bass as bass
import concourse.bacc as bacc
import concourse.mybir as mybir
import concourse.tile as tile
from concourse.bass_utils import run_bass_kernel_spmd

AF = mybir.ActivationFunctionType
BF = mybir.dt.bfloat16
F8 = mybir.dt.float8e4
F32 = mybir.dt.float32
DR = mybir.MatmulPerfMode.DoubleRow
bf16 = ml_dtypes.bfloat16
f8 = ml_dtypes.float8_e4m3

# Problem constants (hardcoded; kernel.py must be self-contained)
B, T, CIN, CH, M, NCLS = 2048, 8, 2048, 1024, 32, 21
NCORES = 8
BL = B // NCORES          # 256 batch rows per core
J = NCLS * M              # 672
P = 128
KB = CIN // P             # 16 contraction blocks over CIN
KP = KB // 2              # 8 DoubleRow pairs over CIN
CHB = CH // P             # 8 blocks over CH
NB = BL // P              # 2 batch chunks of 128
RSK = 128                 # norm-sketch rank
SD, SW, SA, SU = 64.0, 32.0, 16.0, 256.0
JC = [(0, 256), (256, 512), (512, J)]   # dots psum chunks (bank-safe)


def build_nc(debug=False):
    nc = bacc.Bacc("TRN2", target_bir_lowering=False, debug=debug,
                   num_devices=NCORES)

    x8_e = nc.dram_tensor("x8", [P, KB * BL], F8, kind="ExternalInput")
    x8lo_e = nc.dram_tensor("x8lo", [P, KB * BL], F8, kind="ExternalInput")
    wsk8_e = nc.dram_tensor("wsk8", [P, KB * P], F8, kind="ExternalInput")
    d8_e = nc.dram_tensor("d8", [P, KB * J], F8, kind="ExternalInput")
    wv8_e = nc.dram_tensor("wv8", [CHB, P, KB * P], F8, kind="ExternalInput")
    wv8lo_e = nc.dram_tensor("wv8lo", [CHB, P, KB * P], F8, kind="ExternalInput")
    a8_e = nc.dram_tensor("a8", [P, 6 * CH], F8, kind="ExternalInput")
    evwb_e = nc.dram_tensor("evwb", [P, J], F8, kind="ExternalInput")
    bke_e = nc.dram_tensor("bke", [1, J], BF, kind="ExternalInput")
    wout_e = nc.dram_tensor("wout", [P, KB * NCLS], BF, kind="ExternalInput")
    bsk_e = nc.dram_tensor("bsk", [P, 1], F32, kind="ExternalInput")
    bv_e = nc.dram_tensor("bv", [P, CHB], F32, kind="ExternalInput")
    bout_e = nc.dram_tensor("bout", [NCLS, 1], F32, kind="ExternalInput")
    ident_e = nc.dram_tensor("ident", [P, P], BF, kind="ExternalInput")
    out_e = nc.dram_tensor("out", [NCLS, BL], F32, kind="ExternalOutput")

    with tile.TileContext(nc) as tc:
        from contextlib import ExitStack
        with ExitStack() as ctx:
            pers = ctx.enter_context(tc.tile_pool(name="pers", bufs=1))
            pmisc = ctx.enter_context(tc.tile_pool(name="pmisc", bufs=1, space="PSUM"))
            pkv = ctx.enter_context(tc.tile_pool(name="pkv", bufs=2, space="PSUM"))
            pdot = ctx.enter_context(tc.tile_pool(name="pdot", bufs=1, space="PSUM"))
            ptr = ctx.enter_context(tc.tile_pool(name="ptr", bufs=1, space="PSUM"))
            pfe = ctx.enter_context(tc.tile_pool(name="pfe", bufs=1, space="PSUM"))

            # ---- SBUF tiles ----
            x8_s = pers.tile([P, KB, BL], F8, tag="x8")
            x8lo_s = pers.tile([P, KB, BL], F8, tag="x8lo")
            wsk8_s = pers.tile([P, KB, P], F8, tag="wsk8")
            d8_s = pers.tile([P, KB, J], F8, tag="d8")
            wv8_s = pers.tile([P, CHB, KB, P], F8, tag="wv8")
            wv8lo_s = pers.tile([P, CHB, KB, P], F8, tag="wv8lo")
            a8_s = pers.tile([P, 6, CH], F8, tag="a8")
            evwb_s = pers.tile([P, J], F8, tag="evwb")
            bke_s = pers.tile([1, J], BF, tag="bke")
            wout_s = pers.tile([P, KB, NCLS], BF, tag="wout")
            bsk_s = pers.tile([P, 1], F32, tag="bsk")
            bv_s = pers.tile([P, CHB], F32, tag="bv")
            bout_s = pers.tile([NCLS, 1], F32, tag="bout")
            ident_s = pers.tile([P, P], BF, tag="ident")
            ones_s = pers.tile([P, 1], BF, tag="ones")
            ones1_s = pers.tile([1, P], BF, tag="ones1")
            scratch_s = pers.tile([1, 1], F32, tag="scratch")
            sk_s = pers.tile([P, BL], BF, tag="sk")
            sksq_s = pers.tile([P, BL], BF, tag="sksq")
            hv_s = pers.tile([P, CHB, BL], BF, tag="hv")
            hfe_s = pers.tile([P, CHB, BL], BF, tag="hfe")
            e_s = pers.tile([P, NB, J], BF, tag="e")
            tmp_s = pers.tile([P, J], BF, tag="tmp")
            u_s = pers.tile([P, NB, J], BF, tag="u")
            ut_s = pers.tile([P, 6, BL], F8, tag="ut")
            rinv_s = pers.tile([P, NB], F32, tag="rinv")
            rs1_s = pers.tile([P, NB], F32, tag="rs1")
            rs2_s = pers.tile([P, NB], F32, tag="rs2")
            magic_s = pers.tile([P, 1], mybir.dt.int32, tag="magic")
            s_s = pers.tile([P, NB * NCLS], F32, tag="s")
            num_s = pers.tile([P, NB * NCLS], F32, tag="num")
            sinv_s = pers.tile([P, NB * NCLS], F32, tag="sinv")
            t_s = pers.tile([P, NB * NCLS], F32, tag="t")
            g_s = pers.tile([P, NB * NCLS], F32, tag="g")
            gg_s = pers.tile([P, NB], F32, tag="gg")
            ginv_s = pers.tile([P, NB], F32, tag="ginv")
            c1_s = pers.tile([P, NB * NCLS], F32, tag="c1")
            c_s = pers.tile([P, NB * NCLS], F32, tag="c")
            out_sb = pers.tile([NCLS, BL], F32, tag="outsb")

            # ---- setup: memsets + pin the Exp ACT table before any evict ----
            nc.vector.memset(ones_s[:], 1.0)
            nc.vector.memset(ones1_s[:], 1.0)
            nc.vector.memset(magic_s[:], 0x5f3759df)
            nc.vector.memset(ut_s[:], 0.0)        # zero jb-5 pad partitions
            nc.vector.memset(scratch_s[:], 1.0)
            nc.scalar.activation(scratch_s[:], scratch_s[:], AF.Exp)

            # ---- DMA schedule (sync queue; DMA device is the critical
            # resource at ~22.6us busy). wv pairs last: each gates only one
            # v pass. ----
            nc.sync.dma_start(x8_s[:], x8_e.ap())
            nc.sync.dma_start(wsk8_s[:], wsk8_e.ap())
            nc.sync.dma_start(bke_s[:], bke_e.ap())
            nc.sync.dma_start(evwb_s[:], evwb_e.ap())
            nc.sync.dma_start(d8_s[:], d8_e.ap())
            nc.sync.dma_start(a8_s[:], a8_e.ap())
            nc.sync.dma_start(x8lo_s[:], x8lo_e.ap())
            nc.sync.dma_start(wout_s[:], wout_e.ap())
            for o in range(CHB):
                nc.sync.dma_start(wv8_s[:, o, :, :], wv8_e.ap()[o])
                nc.sync.dma_start(wv8lo_s[:, o, :, :], wv8lo_e.ap()[o])
            nc.gpsimd.dma_start(bsk_s[:], bsk_e.ap())
            nc.gpsimd.dma_start(bv_s[:], bv_e.ap())
            nc.gpsimd.dma_start(bout_s[:], bout_e.ap())
            nc.gpsimd.dma_start(ident_s[:], ident_e.ap())

            # ---- phase 1: norm sketch: sk = 64*S*k, rinv = 1/||sk|| ----
            ps = pkv.tile([P, BL], F32, tag="pkv")
            for p in range(KP):
                nc.tensor.matmul(ps[:], wsk8_s[:, 2 * p:2 * p + 2, :],
                                 x8_s[:, 2 * p:2 * p + 2, :],
                                 start=(p == 0), stop=(p == KP - 1),
                                 perf_mode=DR)
            nc.scalar.activation(sk_s[:], ps[:], AF.Identity, bias=bsk_s[:])
            nc.vector.tensor_mul(sksq_s[:], sk_s[:], sk_s[:])
            ps2 = pmisc.tile([P, NB], F32, tag="misc")
            for bc in range(NB):
                nc.tensor.matmul(ps2[:, bc:bc + 1],
                                 sksq_s[:, bc * P:(bc + 1) * P], ones_s[:],
                                 start=True, stop=True)
                sq = rs1_s[:, bc:bc + 1]
                nc.vector.tensor_copy(sq, ps2[:, bc:bc + 1])
                y = rinv_s[:, bc:bc + 1]
                nc.vector.tensor_scalar(
                    y.bitcast(mybir.dt.int32), sq.bitcast(mybir.dt.int32),
                    1, None, op0=mybir.AluOpType.logical_shift_right)
                nc.vector.tensor_tensor(
                    out=y.bitcast(mybir.dt.int32), in0=magic_s[:],
                    in1=y.bitcast(mybir.dt.int32),
                    op=mybir.AluOpType.subtract)
                for _ in range(2):
                    t1 = rs2_s[:, bc:bc + 1]
                    nc.vector.tensor_mul(t1, y, y)
                    nc.vector.tensor_mul(t1, t1, sq)
                    nc.vector.tensor_scalar(t1, t1, -0.5, 1.5,
                                            op0=mybir.AluOpType.mult,
                                            op1=mybir.AluOpType.add)
                    nc.vector.tensor_mul(y, y, t1)

            # ---- phase 2: dots = x8 @ D8 (+bkE), chunk-major fp8 DR ----
            def dots(bc):
                psd = pdot.tile([P, J], F32, tag="pdot")
                for (c0, c1) in JC:
                    for p in range(KP):
                        nc.tensor.matmul(
                            psd[:, c0:c1],
                            x8_s[:, 2 * p:2 * p + 2, bc * P:bc * P + P],
                            d8_s[:, 2 * p:2 * p + 2, c0:c1],
                            start=(p == 0), stop=False, perf_mode=DR)
                    # += bkE (K=1 rank-1 broadcast matmul closes the group)
                    nc.tensor.matmul(psd[:, c0:c1], ones1_s[:],
                                     bke_s[:, c0:c1], start=False, stop=True)
                nc.scalar.activation(e_s[:, bc, 0:512], psd[:, 0:512], AF.Exp,
                                     scale=rinv_s[:, bc:bc + 1])
                nc.scalar.activation(e_s[:, bc, 512:J], psd[:, 512:J], AF.Exp,
                                     scale=rinv_s[:, bc:bc + 1])

            def softmax_chain():
                # both batch chunks in one pass: [P, NB*J] wide DVE ops so
                # per-op seq/sem overhead is paid once, not per chunk
                e3 = e_s[:].rearrange("p n (c m) -> p (n c) m", m=M)
                nc.vector.reduce_sum(s_s[:], e3, axis=mybir.AxisListType.X)
                evwb_b = bass.AP(evwb_s.tensor, evwb_s[:].offset,
                                 [evwb_s[:].ap[0], [0, NB]] + evwb_s[:].ap[1:])
                nc.vector.tensor_mul(u_s[:], e_s[:], evwb_b)
                nc.vector.reduce_sum(
                    num_s[:], u_s[:].rearrange("p n (c m) -> p (n c) m", m=M),
                    axis=mybir.AxisListType.X)
                nc.vector.reciprocal(sinv_s[:], s_s[:])
                nc.vector.tensor_mul(t_s[:], num_s[:], sinv_s[:])
                nc.scalar.activation(g_s[:], t_s[:], AF.Exp)
                nc.vector.reduce_sum(
                    gg_s[:], g_s[:].rearrange("p (n c) -> p n c", c=NCLS),
                    axis=mybir.AxisListType.X)
                nc.vector.reciprocal(ginv_s[:], gg_s[:])
                # fold the u scale SU into ginv: c = g*sinv * (SU/G)
                nc.vector.tensor_scalar(ginv_s[:], ginv_s[:], SU, None,
                                        op0=mybir.AluOpType.mult)
                nc.vector.tensor_mul(c1_s[:], g_s[:], sinv_s[:])
                ginv_b = bass.AP(ginv_s.tensor, ginv_s[:].offset,
                                 ginv_s[:].ap + [[0, NCLS]])
                nc.vector.tensor_mul(
                    c_s[:].rearrange("p (n c) -> p n c", c=NCLS),
                    c1_s[:].rearrange("p (n c) -> p n c", c=NCLS), ginv_b)
                c_b = bass.AP(c_s.tensor, c_s[:].offset,
                              c_s[:].ap + [[0, M]])
                u3 = e_s[:].rearrange("p n (c m) -> p (n c) m", m=M)
                nc.vector.tensor_mul(
                    u_s[:].rearrange("p n (c m) -> p (n c) m", m=M), u3, c_b)

            # ---- out accumulator: block matmuls emitted as inputs land ----
            pso = pmisc.tile([NCLS, BL], F32, tag="misc")
            n_out_mm = [0]

            def out_mm(h_s, i):
                kb = i if h_s is hv_s else CHB + i
                nc.tensor.matmul(pso[:], wout_s[:, kb, :], h_s[:, i, :],
                                 start=(n_out_mm[0] == 0),
                                 stop=(n_out_mm[0] == KB - 1))
                n_out_mm[0] += 1

            def v_block(o):
                ps = pkv.tile([P, BL], F32, tag="pkv")
                n = 0
                for (wt, xt) in ((wv8_s, x8_s), (wv8_s, x8lo_s),
                                 (wv8lo_s, x8_s)):
                    for p in range(KP):
                        nc.tensor.matmul(ps[:], wt[:, o, 2 * p:2 * p + 2, :],
                                         xt[:, 2 * p:2 * p + 2, :],
                                         start=(n == 0), stop=(n == 3 * KP - 1),
                                         perf_mode=DR)
                        n += 1
                nc.scalar.activation(hv_s[:, o, :], ps[:], AF.Relu,
                                     scale=1.0 / SW, bias=bv_s[:, o:o + 1])

            def transpose_u(bc):
                def tgroup(grp):
                    pst = ptr.tile([P, 3 * P], BF, tag="ptr")
                    for t, jb in enumerate(grp):
                        w = P if jb < 5 else J - 5 * P
                        nc.tensor.transpose(
                            pst[:w, t * P:(t + 1) * P],
                            u_s[:, bc, jb * P:jb * P + w],
                            ident_s[:])
                    n = sum(1 for jb in grp if jb < 5)
                    base = ut_s[:, grp[0], bc * P:bc * P + P]
                    dst = bass.AP(ut_s.tensor, base.offset,
                                  [base.ap[0], [BL, n], base.ap[1]])
                    nc.vector.tensor_copy(
                        dst, pst[:, 0:n * P].rearrange("p (n q) -> p n q", q=P))
                    if n < len(grp):
                        jb = grp[n]
                        w = J - 5 * P
                        nc.scalar.activation(
                            ut_s[:w, jb, bc * P:bc * P + P],
                            pst[:w, n * P:(n + 1) * P], AF.Identity)
                tgroup((0, 1, 2))
                tgroup((3, 4, 5))

            def fe_half(bc):
                # fE for one batch half: independent [P,128] psum groups so
                # each half runs as soon as its transpose lands
                sl = slice(bc * P, (bc + 1) * P)
                for o in range(CHB):
                    acc = pfe.tile([P, P], F32, tag=f"pfe{o % 2}")
                    for t in range(3):
                        nc.tensor.matmul(acc[:], a8_s[:, 2 * t:2 * t + 2,
                                                      o * P:(o + 1) * P],
                                         ut_s[:, 2 * t:2 * t + 2, sl],
                                         start=(t == 0), stop=(t == 2),
                                         perf_mode=DR)
                    dst = hfe_s[:, o, sl]
                    if o % 2 == 0:
                        nc.scalar.activation(dst, acc[:], AF.Relu,
                                             scale=1.0 / (SA * SU))
                    else:
                        nc.vector.tensor_scalar(dst, acc[:],
                                                1.0 / (SA * SU), 0.0,
                                                op0=mybir.AluOpType.mult,
                                                op1=mybir.AluOpType.max)

            # ---- main interleave: the whole dots/softmax/transpose/fE chain
            # runs before the wv stream thickens; v blocks then track DMA ----
            dots(0)
            dots(1)
            softmax_chain()
            transpose_u(0)
            fe_half(0)
            v_block(0)
            transpose_u(1)
            fe_half(1)
            v_block(1)
            for i in range(CHB):
                out_mm(hfe_s, i)
            for o in range(2, CHB):
                v_block(o)
                out_mm(hv_s, o - 2)
            out_mm(hv_s, CHB - 2)
            out_mm(hv_s, CHB - 1)

            # ---- +bout, DMA out ----
            nc.vector.tensor_scalar_add(out_sb[:], pso[:], bout_s[:, 0:1])
            nc.sync.dma_start(out_e.ap(), out_sb[:])

    nc.compile()
    return nc


def host_prep(x, static_feat, Wk, bk, Wv, bv, WEk, bEk, WEv, bEv, Ww, bw,
              Wout, bout):
    """Host-side fp32 precompute, fp8/bf16 quantization, per-core input maps."""
    EPS = 1e-8
    f32 = np.float32
    x = np.asarray(x, f32)
    static_feat = np.asarray(static_feat, f32)
    Wk, bk = np.asarray(Wk, f32), np.asarray(bk, f32)
    Wv, bv = np.asarray(Wv, f32), np.asarray(bv, f32)
    Wout, bout = np.asarray(Wout, f32), np.asarray(bout, f32)

    Ek = np.einsum('oc,ncm->nom', np.asarray(WEk, f32), static_feat,
                   optimize=True) + np.asarray(bEk, f32)[None, :, None]
    Ev = np.einsum('oc,ncm->nom', np.asarray(WEv, f32), static_feat,
                   optimize=True) + np.asarray(bEv, f32)[None, :, None]
    Ekn = Ek / np.maximum(np.linalg.norm(Ek, axis=1, keepdims=True), EPS)
    Ekn_mat = Ekn.transpose(1, 0, 2).reshape(CH, J)          # [CH, 672]
    A_mat = Ev.transpose(0, 2, 1).reshape(J, CH)             # [672, CH]
    evwb = np.einsum('nom,o->nm', Ev, np.asarray(Ww, f32)[0]).reshape(J)

    # norm sketch + folded dots
    S = np.random.RandomState(0).randn(RSK, CH).astype(f32) / np.sqrt(RSK)
    W_sk = S @ Wk                                            # [128, CIN]
    b_sk = S @ bk
    D = Wk.T @ Ekn_mat                                       # [CIN, J]
    bkE = bk @ Ekn_mat                                       # [J]

    def cinlayout(w, width):    # [CIN, width] -> [P, KB*width]
        return np.ascontiguousarray(
            w.reshape(KB, P, width).transpose(1, 0, 2).reshape(P, KB * width))

    wsk8_h = cinlayout((W_sk.T * SD).astype(f8), P)
    d8_h = cinlayout((D * SD).astype(f8), J)

    def wlayout(w):     # [CIN, OCH] f8 -> dram [OCH/P, P, KB*P]
        och = w.shape[1]
        return np.ascontiguousarray(
            w.reshape(KB, P, och // P, P).transpose(2, 1, 0, 3)
            .reshape(och // P, P, KB * P))

    wv_s = Wv.T * SW
    wv8_f = wv_s.astype(f8)
    wv8lo_f = (wv_s - wv8_f.astype(f32)).astype(f8)
    wv8_h = wlayout(wv8_f)
    wv8lo_h = wlayout(wv8lo_f)

    a_pad = np.zeros((6 * P, CH), f32)
    a_pad[:J] = A_mat * SA
    a8_h = np.ascontiguousarray(
        a_pad.astype(f8).reshape(6, P, CH).transpose(1, 0, 2).reshape(P, 6 * CH))
    evwb_h = np.ascontiguousarray(
        np.broadcast_to(evwb.astype(f8)[None, :], (P, J)))
    bke_h = (bkE * SD).astype(bf16).reshape(1, J)
    wout_h = np.ascontiguousarray(
        Wout.T.reshape(KB, P, NCLS).transpose(1, 0, 2).reshape(
            P, KB * NCLS).astype(bf16))
    bsk_h = np.ascontiguousarray((b_sk * SD).reshape(P, 1))
    bv_h = np.ascontiguousarray(bv.reshape(CHB, P).T)
    bout_h = bout.reshape(NCLS, 1)
    ident_h = np.eye(P, dtype=bf16)

    xT = np.ascontiguousarray(x[:, -1, :].T)                 # [CIN, B]
    x8_f = xT.astype(f8)
    x8lo_f = (xT - x8_f.astype(f32)).astype(f8)

    shared = dict(wsk8=wsk8_h, d8=d8_h, wv8=wv8_h, wv8lo=wv8lo_h, a8=a8_h,
                  evwb=evwb_h, bke=bke_h, wout=wout_h, bsk=bsk_h, bv=bv_h,
                  bout=bout_h, ident=ident_h)
    in_maps = []
    for c in range(NCORES):
        sl = slice(c * BL, (c + 1) * BL)

        def xlayout(xf):
            return np.ascontiguousarray(
                xf[:, sl].reshape(KB, P, BL).transpose(1, 0, 2)
                .reshape(P, KB * BL))
        in_maps.append(dict(x8=xlayout(x8_f), x8lo=xlayout(x8lo_f), **shared))
    return in_maps


_NC_CACHE = {}


def get_nc(debug=False):
    if debug not in _NC_CACHE:
        _NC_CACHE[debug] = build_nc(debug=debug)
    return _NC_CACHE[debug]


def kernel(**inputs) -> np.ndarray:
    nc = get_nc()
    in_maps = host_prep(**inputs)
    res = run_bass_kernel_spmd(nc, in_maps, list(range(NCORES)))
    out = np.empty((B, NCLS, 1), dtype=np.float32)
    for c in range(NCORES):
        out[c * BL:(c + 1) * BL, :, 0] = res.results[c]["out"].T
    return out


# revision 29
# speedup vs baseline: 1.5037x; 1.0355x over previous
"""Trainium2 Bass kernel for nn_Colar_static (retrieval_knn).

Sharding: data-parallel over batch B=2048 across 8 NeuronCores (256 rows each).
Weights/exemplars replicated, precomputed + quantized on host.

Design (vs the bf16 baseline at 53.3us):
  * Every large matmul is fp8e4m3 with the DoubleRow perf mode (K=256 per
    instruction, 0.5 cycles/row) -> 4x bf16 MAC rate and 1-byte weights
    (the kernel is DMA-bound: all DMA serializes at ~332 GB/s).
  * dots = x @ (Wk^T Ekn) directly: Wk is folded into the exemplars on the
    host, so the 2MB Wk and 0.7MB Ekn never ship; only D8 [CIN,672] (1.4MB).
  * ||k|| (softmax temperature only) via a random sketch: ||S k|| with
    S [128,1024] Gaussian, W_sk = S Wk [128, CIN] fp8 (0.25MB). The 5% norm
    error is invisible downstream (validated: rel err 3.21e-3, same as the
    exact-norm pipeline, because cos logits are tiny and softmax-smoothed).
  * v  = x8@Wv8hi + x8lo@Wv8hi + x8@Wv8lo   3-pass residual-compensated fp8
    (v dominates the output; plain fp8 fails at 3.3e-2).
  * fE = A8^T @ ut8 (fp8 DR);  out = Wout^T @ [hv;hfe] in bf16 (tiny).

Scales (all folded, no extra device work): D,W_sk x64; Wv x32; A x16; u x256.
The sketch scale cancels: rinv = rsqrt(sum((64 S k)^2)) = 1/(64||Sk||) and
dots are x64, so exp(dots*rinv) = exp(cos).

Rel err vs fp32 reference ~3.2e-3 (numpy-sim validated; gate is 2e-2).
"""

import numpy as np
import ml_dtypes

import concourse.bass as bass
import concourse.bacc as bacc
import concourse.mybir as mybir
import concourse.tile as tile
from concourse.bass_utils import run_bass_kernel_spmd

AF = mybir.ActivationFunctionType
BF = mybir.dt.bfloat16
F8 = mybir.dt.float8e4
F32 = mybir.dt.float32
DR = mybir.MatmulPerfMode.DoubleRow
bf16 = ml_dtypes.bfloat16
f8 = ml_dtypes.float8_e4m3

# Problem constants (hardcoded; kernel.py must be self-contained)
B, T, CIN, CH, M, NCLS = 2048, 8, 2048, 1024, 32, 21
NCORES = 8
BL = B // NCORES          # 256 batch rows per core
J = NCLS * M              # 672
P = 128
KB = CIN // P             # 16 contraction blocks over CIN
KP = KB // 2              # 8 DoubleRow pairs over CIN
CHB = CH // P             # 8 blocks over CH
NB = BL // P              # 2 batch chunks of 128
RSK = 128                 # norm-sketch rank
SD, SW, SA, SU = 64.0, 32.0, 16.0, 256.0
JC = [(0, 256), (256, 512), (512, J)]   # dots psum chunks (bank-safe)


def build_nc(debug=False):
    nc = bacc.Bacc("TRN2", target_bir_lowering=False, debug=debug,
                   num_devices=NCORES)

    x8_e = nc.dram_tensor("x8", [P, KB * BL], F8, kind="ExternalInput")
    x8lo_e = nc.dram_tensor("x8lo", [P, KB * BL], F8, kind="ExternalInput")
    wsk8_e = nc.dram_tensor("wsk8", [P, KB * P], F8, kind="ExternalInput")
    d8_e = nc.dram_tensor("d8", [P, KB * J], F8, kind="ExternalInput")
    wv8_e = nc.dram_tensor("wv8", [CHB, P, KB * P], F8, kind="ExternalInput")
    wv8lo_e = nc.dram_tensor("wv8lo", [CHB, P, KB * P], F8, kind="ExternalInput")
    a8_e = nc.dram_tensor("a8", [P, 6 * CH], F8, kind="ExternalInput")
    evwb_e = nc.dram_tensor("evwb", [P, J], F8, kind="ExternalInput")
    bke_e = nc.dram_tensor("bke", [1, J], BF, kind="ExternalInput")
    wout_e = nc.dram_tensor("wout", [P, KB * NCLS], BF, kind="ExternalInput")
    bsk_e = nc.dram_tensor("bsk", [P, 1], F32, kind="ExternalInput")
    bv_e = nc.dram_tensor("bv", [P, CHB], F32, kind="ExternalInput")
    bout_e = nc.dram_tensor("bout", [NCLS, 1], F32, kind="ExternalInput")
    ident_e = nc.dram_tensor("ident", [P, P], BF, kind="ExternalInput")
    out_e = nc.dram_tensor("out", [NCLS, BL], F32, kind="ExternalOutput")

    with tile.TileContext(nc) as tc:
        from contextlib import ExitStack
        with ExitStack() as ctx:
            pers = ctx.enter_context(tc.tile_pool(name="pers", bufs=1))
            pmisc = ctx.enter_context(tc.tile_pool(name="pmisc", bufs=1, space="PSUM"))
            pkv = ctx.enter_context(tc.tile_pool(name="pkv", bufs=2, space="PSUM"))
            pdot = ctx.enter_context(tc.tile_pool(name="pdot", bufs=1, space="PSUM"))
            ptr = ctx.enter_context(tc.tile_pool(name="ptr", bufs=1, space="PSUM"))
            pfe = ctx.enter_context(tc.tile_pool(name="pfe", bufs=1, space="PSUM"))

            # ---- SBUF tiles ----
            x8_s = pers.tile([P, KB, BL], F8, tag="x8")
            x8lo_s = pers.tile([P, KB, BL], F8, tag="x8lo")
            wsk8_s = pers.tile([P, KB, P], F8, tag="wsk8")
            d8_s = pers.tile([P, KB, J], F8, tag="d8")
            wv8_s = pers.tile([P, CHB, KB, P], F8, tag="wv8")
            wv8lo_s = pers.tile([P, CHB, KB, P], F8, tag="wv8lo")
            a8_s = pers.tile([P, 6, CH], F8, tag="a8")
            evwb_s = pers.tile([P, J], F8, tag="evwb")
            bke_s = pers.tile([1, J], BF, tag="bke")
            wout_s = pers.tile([P, KB, NCLS], BF, tag="wout")
            bsk_s = pers.tile([P, 1], F32, tag="bsk")
            bv_s = pers.tile([P, CHB], F32, tag="bv")
            bout_s = pers.tile([NCLS, 1], F32, tag="bout")
            ident_s = pers.tile([P, P], BF, tag="ident")
            ones_s = pers.tile([P, 1], BF, tag="ones")
            ones1_s = pers.tile([1, P], BF, tag="ones1")
            scratch_s = pers.tile([1, 1], F32, tag="scratch")
            sk_s = pers.tile([P, BL], BF, tag="sk")
            sksq_s = pers.tile([P, BL], BF, tag="sksq")
            hv_s = pers.tile([P, CHB, BL], BF, tag="hv")
            hfe_s = pers.tile([P, CHB, BL], BF, tag="hfe")
            e_s = pers.tile([P, NB, J], BF, tag="e")
            tmp_s = pers.tile([P, J], BF, tag="tmp")
            u_s = pers.tile([P, NB, J], BF, tag="u")
            ut_s = pers.tile([P, 6, BL], F8, tag="ut")
            rinv_s = pers.tile([P, NB], F32, tag="rinv")
            rs1_s = pers.tile([P, NB], F32, tag="rs1")
            rs2_s = pers.tile([P, NB], F32, tag="rs2")
            magic_s = pers.tile([P, 1], mybir.dt.int32, tag="magic")
            s_s = pers.tile([P, NB * NCLS], F32, tag="s")
            num_s = pers.tile([P, NB * NCLS], F32, tag="num")
            sinv_s = pers.tile([P, NB * NCLS], F32, tag="sinv")
            t_s = pers.tile([P, NB * NCLS], F32, tag="t")
            g_s = pers.tile([P, NB * NCLS], F32, tag="g")
            gg_s = pers.tile([P, NB], F32, tag="gg")
            ginv_s = pers.tile([P, NB], F32, tag="ginv")
            c1_s = pers.tile([P, NB * NCLS], F32, tag="c1")
            c_s = pers.tile([P, NB * NCLS], F32, tag="c")
            out_sb = pers.tile([NCLS, BL], F32, tag="outsb")

            # ---- setup: memsets + pin the Exp ACT table before any evict ----
            nc.vector.memset(ones_s[:], 1.0)
            nc.vector.memset(ones1_s[:], 1.0)
            nc.vector.memset(magic_s[:], 0x5f3759df)
            nc.vector.memset(ut_s[:], 0.0)        # zero jb-5 pad partitions
            nc.vector.memset(scratch_s[:], 1.0)
            nc.scalar.activation(scratch_s[:], scratch_s[:], AF.Exp)

            # ---- DMA schedule (sync queue; DMA device is the critical
            # resource at ~22.6us busy). wv pairs last: each gates only one
            # v pass. ----
            nc.sync.dma_start(x8_s[:], x8_e.ap())
            nc.sync.dma_start(wsk8_s[:], wsk8_e.ap())
            nc.sync.dma_start(bke_s[:], bke_e.ap())
            nc.sync.dma_start(evwb_s[:], evwb_e.ap())
            nc.sync.dma_start(d8_s[:], d8_e.ap())
            nc.sync.dma_start(x8lo_s[:], x8lo_e.ap())
            nc.sync.dma_start(wout_s[:], wout_e.ap())
            for o in range(CHB):
                nc.sync.dma_start(wv8_s[:, o, :, :], wv8_e.ap()[o])
                nc.sync.dma_start(wv8lo_s[:, o, :, :], wv8lo_e.ap()[o])
                if o == 1:
                    nc.sync.dma_start(a8_s[:], a8_e.ap())
            nc.gpsimd.dma_start(bsk_s[:], bsk_e.ap())
            nc.gpsimd.dma_start(bv_s[:], bv_e.ap())
            nc.gpsimd.dma_start(bout_s[:], bout_e.ap())
            nc.gpsimd.dma_start(ident_s[:], ident_e.ap())

            # ---- phase 1: norm sketch: sk = 64*S*k, rinv = 1/||sk|| ----
            ps = pkv.tile([P, BL], F32, tag="pkv")
            for p in range(KP):
                nc.tensor.matmul(ps[:], wsk8_s[:, 2 * p:2 * p + 2, :],
                                 x8_s[:, 2 * p:2 * p + 2, :],
                                 start=(p == 0), stop=(p == KP - 1),
                                 perf_mode=DR)
            nc.scalar.activation(sk_s[:], ps[:], AF.Identity, bias=bsk_s[:])
            nc.vector.tensor_mul(sksq_s[:], sk_s[:], sk_s[:])
            ps2 = pmisc.tile([P, NB], F32, tag="misc")
            for bc in range(NB):
                nc.tensor.matmul(ps2[:, bc:bc + 1],
                                 sksq_s[:, bc * P:(bc + 1) * P], ones_s[:],
                                 start=True, stop=True)
                sq = rs1_s[:, bc:bc + 1]
                nc.vector.tensor_copy(sq, ps2[:, bc:bc + 1])
                y = rinv_s[:, bc:bc + 1]
                nc.vector.tensor_scalar(
                    y.bitcast(mybir.dt.int32), sq.bitcast(mybir.dt.int32),
                    1, None, op0=mybir.AluOpType.logical_shift_right)
                nc.vector.tensor_tensor(
                    out=y.bitcast(mybir.dt.int32), in0=magic_s[:],
                    in1=y.bitcast(mybir.dt.int32),
                    op=mybir.AluOpType.subtract)
                for _ in range(2):
                    t1 = rs2_s[:, bc:bc + 1]
                    nc.vector.tensor_mul(t1, y, y)
                    nc.vector.tensor_mul(t1, t1, sq)
                    nc.vector.tensor_scalar(t1, t1, -0.5, 1.5,
                                            op0=mybir.AluOpType.mult,
                                            op1=mybir.AluOpType.add)
                    nc.vector.tensor_mul(y, y, t1)

            # ---- phase 2: dots = x8 @ D8 (+bkE), chunk-major fp8 DR ----
            def dots(bc):
                psd = pdot.tile([P, J], F32, tag="pdot")
                for (c0, c1) in JC:
                    for p in range(KP):
                        nc.tensor.matmul(
                            psd[:, c0:c1],
                            x8_s[:, 2 * p:2 * p + 2, bc * P:bc * P + P],
                            d8_s[:, 2 * p:2 * p + 2, c0:c1],
                            start=(p == 0), stop=False, perf_mode=DR)
                    # += bkE (K=1 rank-1 broadcast matmul closes the group)
                    nc.tensor.matmul(psd[:, c0:c1], ones1_s[:],
                                     bke_s[:, c0:c1], start=False, stop=True)
                nc.scalar.activation(e_s[:, bc, 0:512], psd[:, 0:512], AF.Exp,
                                     scale=rinv_s[:, bc:bc + 1])
                nc.scalar.activation(e_s[:, bc, 512:J], psd[:, 512:J], AF.Exp,
                                     scale=rinv_s[:, bc:bc + 1])

            def softmax_chain(bc, eng):
                # bc0 runs on DVE, bc1 on gpsimd: the two chains execute in
                # parallel so u1 lands ~2us earlier
                e_sl = e_s[:, bc, :]
                e3 = e_sl.rearrange("p (n m) -> p n m", m=M)
                ncls_sl = slice(bc * NCLS, (bc + 1) * NCLS)
                s2 = s_s[:, ncls_sl]
                eng.reduce_sum(s2, e3, axis=mybir.AxisListType.X)
                u_tmp = u_s[:, bc, :]
                eng.tensor_mul(u_tmp, e_sl, evwb_s[:])
                eng.reduce_sum(num_s[:, ncls_sl],
                               u_tmp.rearrange("p (n m) -> p n m", m=M),
                               axis=mybir.AxisListType.X)
                eng.reciprocal(sinv_s[:, ncls_sl], s2)
                eng.tensor_mul(t_s[:, ncls_sl], num_s[:, ncls_sl],
                               sinv_s[:, ncls_sl])
                nc.scalar.activation(g_s[:, ncls_sl], t_s[:, ncls_sl], AF.Exp)
                eng.reduce_sum(gg_s[:, bc:bc + 1], g_s[:, ncls_sl],
                               axis=mybir.AxisListType.X)
                eng.reciprocal(ginv_s[:, bc:bc + 1], gg_s[:, bc:bc + 1])
                # fold the u scale SU into ginv: c = g*sinv * (SU/G)
                eng.tensor_scalar(ginv_s[:, bc:bc + 1], ginv_s[:, bc:bc + 1],
                                  SU, None, op0=mybir.AluOpType.mult)
                eng.tensor_mul(c1_s[:, ncls_sl], g_s[:, ncls_sl],
                               sinv_s[:, ncls_sl])
                eng.tensor_scalar_mul(c_s[:, ncls_sl], c1_s[:, ncls_sl],
                                      ginv_s[:, bc:bc + 1])
                c_b = bass.AP(c_s.tensor, c_s[:, ncls_sl].offset,
                              c_s[:, ncls_sl].ap + [[0, M]])
                u3 = u_s[:, bc, :].rearrange("p (n m) -> p n m", m=M)
                eng.tensor_mul(u3, e3, c_b)

            # ---- out accumulator: block matmuls emitted as inputs land ----
            pso = pmisc.tile([NCLS, BL], F32, tag="misc")
            n_out_mm = [0]

            def out_mm(h_s, i):
                kb = i if h_s is hv_s else CHB + i
                nc.tensor.matmul(pso[:], wout_s[:, kb, :], h_s[:, i, :],
                                 start=(n_out_mm[0] == 0),
                                 stop=(n_out_mm[0] == KB - 1))
                n_out_mm[0] += 1

            def v_block(o):
                ps = pkv.tile([P, BL], F32, tag="pkv")
                n = 0
                for (wt, xt) in ((wv8_s, x8_s), (wv8_s, x8lo_s),
                                 (wv8lo_s, x8_s)):
                    for p in range(KP):
                        nc.tensor.matmul(ps[:], wt[:, o, 2 * p:2 * p + 2, :],
                                         xt[:, 2 * p:2 * p + 2, :],
                                         start=(n == 0), stop=(n == 3 * KP - 1),
                                         perf_mode=DR)
                        n += 1
                nc.scalar.activation(hv_s[:, o, :], ps[:], AF.Relu,
                                     scale=1.0 / SW, bias=bv_s[:, o:o + 1])

            def transpose_u(bc):
                def tgroup(grp):
                    pst = ptr.tile([P, 3 * P], BF, tag="ptr")
                    for t, jb in enumerate(grp):
                        w = P if jb < 5 else J - 5 * P
                        nc.tensor.transpose(
                            pst[:w, t * P:(t + 1) * P],
                            u_s[:, bc, jb * P:jb * P + w],
                            ident_s[:])
                    n = sum(1 for jb in grp if jb < 5)
                    base = ut_s[:, grp[0], bc * P:bc * P + P]
                    dst = bass.AP(ut_s.tensor, base.offset,
                                  [base.ap[0], [BL, n], base.ap[1]])
                    nc.vector.tensor_copy(
                        dst, pst[:, 0:n * P].rearrange("p (n q) -> p n q", q=P))
                    if n < len(grp):
                        jb = grp[n]
                        w = J - 5 * P
                        nc.scalar.activation(
                            ut_s[:w, jb, bc * P:bc * P + P],
                            pst[:w, n * P:(n + 1) * P], AF.Identity)
                tgroup((0, 1, 2))
                tgroup((3, 4, 5))

            def fe_all():
                for o in range(CHB):
                    acc = pfe.tile([P, BL], F32, tag=f"pfe{o % 2}")
                    for t in range(3):
                        nc.tensor.matmul(acc[:], a8_s[:, 2 * t:2 * t + 2,
                                                      o * P:(o + 1) * P],
                                         ut_s[:, 2 * t:2 * t + 2, :],
                                         start=(t == 0), stop=(t == 2),
                                         perf_mode=DR)
                    dst = hfe_s[:, o, :]
                    if o % 2 == 0:
                        nc.scalar.activation(dst, acc[:], AF.Relu,
                                             scale=1.0 / (SA * SU))
                    else:
                        nc.vector.tensor_scalar(dst, acc[:],
                                                1.0 / (SA * SU), 0.0,
                                                op0=mybir.AluOpType.mult,
                                                op1=mybir.AluOpType.max)

            # ---- main interleave: the whole dots/softmax/transpose/fE chain
            # runs before the wv stream thickens; v blocks then track DMA ----
            dots(0)
            softmax_chain(0, nc.vector)
            v_block(0)
            dots(1)
            softmax_chain(1, nc.vector)
            v_block(1)
            out_mm(hv_s, 0)
            transpose_u(0)
            transpose_u(1)
            fe_all()
            for i in range(CHB):
                out_mm(hfe_s, i)
            for o in range(2, CHB):
                v_block(o)
                out_mm(hv_s, o - 1)
            out_mm(hv_s, CHB - 1)

            # ---- +bout, DMA out ----
            nc.vector.tensor_scalar_add(out_sb[:], pso[:], bout_s[:, 0:1])
            nc.sync.dma_start(out_e.ap(), out_sb[:])

    nc.compile()
    return nc


def host_prep(x, static_feat, Wk, bk, Wv, bv, WEk, bEk, WEv, bEv, Ww, bw,
              Wout, bout):
    """Host-side fp32 precompute, fp8/bf16 quantization, per-core input maps."""
    EPS = 1e-8
    f32 = np.float32
    x = np.asarray(x, f32)
    static_feat = np.asarray(static_feat, f32)
    Wk, bk = np.asarray(Wk, f32), np.asarray(bk, f32)
    Wv, bv = np.asarray(Wv, f32), np.asarray(bv, f32)
    Wout, bout = np.asarray(Wout, f32), np.asarray(bout, f32)

    Ek = np.einsum('oc,ncm->nom', np.asarray(WEk, f32), static_feat,
                   optimize=True) + np.asarray(bEk, f32)[None, :, None]
    Ev = np.einsum('oc,ncm->nom', np.asarray(WEv, f32), static_feat,
                   optimize=True) + np.asarray(bEv, f32)[None, :, None]
    Ekn = Ek / np.maximum(np.linalg.norm(Ek, axis=1, keepdims=True), EPS)
    Ekn_mat = Ekn.transpose(1, 0, 2).reshape(CH, J)          # [CH, 672]
    A_mat = Ev.transpose(0, 2, 1).reshape(J, CH)             # [672, CH]
    evwb = np.einsum('nom,o->nm', Ev, np.asarray(Ww, f32)[0]).reshape(J)

    # norm sketch + folded dots
    S = np.random.RandomState(0).randn(RSK, CH).astype(f32) / np.sqrt(RSK)
    W_sk = S @ Wk                                            # [128, CIN]
    b_sk = S @ bk
    D = Wk.T @ Ekn_mat                                       # [CIN, J]
    bkE = bk @ Ekn_mat                                       # [J]

    def cinlayout(w, width):    # [CIN, width] -> [P, KB*width]
        return np.ascontiguousarray(
            w.reshape(KB, P, width).transpose(1, 0, 2).reshape(P, KB * width))

    wsk8_h = cinlayout((W_sk.T * SD).astype(f8), P)
    d8_h = cinlayout((D * SD).astype(f8), J)

    def wlayout(w):     # [CIN, OCH] f8 -> dram [OCH/P, P, KB*P]
        och = w.shape[1]
        return np.ascontiguousarray(
            w.reshape(KB, P, och // P, P).transpose(2, 1, 0, 3)
            .reshape(och // P, P, KB * P))

    wv_s = Wv.T * SW
    wv8_f = wv_s.astype(f8)
    wv8lo_f = (wv_s - wv8_f.astype(f32)).astype(f8)
    wv8_h = wlayout(wv8_f)
    wv8lo_h = wlayout(wv8lo_f)

    a_pad = np.zeros((6 * P, CH), f32)
    a_pad[:J] = A_mat * SA
    a8_h = np.ascontiguousarray(
        a_pad.astype(f8).reshape(6, P, CH).transpose(1, 0, 2).reshape(P, 6 * CH))
    evwb_h = np.ascontiguousarray(
        np.broadcast_to(evwb.astype(f8)[None, :], (P, J)))
    bke_h = (bkE * SD).astype(bf16).reshape(1, J)
    wout_h = np.ascontiguousarray(
        Wout.T.reshape(KB, P, NCLS).transpose(1, 0, 2).reshape(
            P, KB * NCLS).astype(bf16))
    bsk_h = np.ascontiguousarray((b_sk * SD).reshape(P, 1))
    bv_h = np.ascontiguousarray(bv.reshape(CHB, P).T)
    bout_h = bout.reshape(NCLS, 1)
    ident_h = np.eye(P, dtype=bf16)

    xT = np.ascontiguousarray(x[:, -1, :].T)                 # [CIN, B]
    x8_f = xT.astype(f8)
    x8lo_f = (xT - x8_f.astype(f32)).astype(f8)

    shared = dict(wsk8=wsk8_h, d8=d8_h, wv8=wv8_h, wv8lo=wv8lo_h, a8=a8_h,
                  evwb=evwb_h, bke=bke_h, wout=wout_h, bsk=bsk_h, bv=bv_h,
                  bout=bout_h, ident=ident_h)
    in_maps = []
    for c in range(NCORES):
        sl = slice(c * BL, (c + 1) * BL)

        def xlayout(xf):
            return np.ascontiguousarray(
                xf[:, sl].reshape(KB, P, BL).transpose(1, 0, 2)
                .reshape(P, KB * BL))
        in_maps.append(dict(x8=xlayout(x8_f), x8lo=xlayout(x8lo_f), **shared))
    return in_maps


_NC_CACHE = {}


def get_nc(debug=False):
    if debug not in _NC_CACHE:
        _NC_CACHE[debug] = build_nc(debug=debug)
    return _NC_CACHE[debug]


def kernel(**inputs) -> np.ndarray:
    nc = get_nc()
    in_maps = host_prep(**inputs)
    res = run_bass_kernel_spmd(nc, in_maps, list(range(NCORES)))
    out = np.empty((B, NCLS, 1), dtype=np.float32)
    for c in range(NCORES):
        out[c * BL:(c + 1) * BL, :, 0] = res.results[c]["out"].T
    return out
